# revision 11
# baseline (speedup 1.0000x reference)
"""Trainium2 Bass kernel for nn_LocationDependentClassifier.

Reference computation (for full input x of shape (64, 3, 512, 512) f32):
    top_left = x[:, :, :8, :8].mean(axis=(1, 2, 3))          # (64,)
    pred     = mod(trunc(top_left * 10), 10)                 # int in [0, 10)
    logits   = 10 * one_hot(pred, 10)                        # (64, 10) f32

Only the 8x8 top-left patch of each channel is live: 64*3*8*8 floats (48 KiB)
out of 201 MB. Sharding strategy (pure data parallelism per the hint): the
batch dim is split across the 8 cores, and each core is handed exactly the
bytes it needs -- its 8 images' top-left patches, flattened to (8, 192).

On-device per core (all fp32, all on the DVE; 4-op dependency chain):
    s = reduce_sum(patch_row_b)                              # (8, 1)
    S = (CONST <= s) * 10                                    # (8, 40)
    a = S[:, 0:20] - S[:, 20:40]                             # (8, 20)
    o = a[:, 0:10] + a[:, 10:20]                             # (8, 10)

CONST columns are [LO(20) | HI(20)], LO = [lo1 | lo2], HI = [hi1 | hi2], so
a[:, j] = 10 * ind(lo_j <= s < hi_j) and o sums the positive/negative trunc
branches. Class c fires iff t in [c, c+1) (positive branch; c=0 widens to
[-1, 1)) or t in [c-11, c-10) (negative branch, c >= 1), t = sum * 10/192.
Thresholds are pre-multiplied by 192/10 so the comparison runs on the raw
sum. Every intermediate is an exact small integer in fp32; the only
inexactness is the sum itself (boundary margin ~5 orders above fp32 noise).

The kernel is latency-bound: the runtime's fixed per-execution wrapper
(engine wake + clock-sync rings before the program, a full semaphore sweep
after it) dominates, so the controllable cost is the span from the first
program instruction to the last engine's retirement. To keep that span
minimal:
  - Bass's const-AP memset preamble is skipped (nothing reads it).
  - Both bass all-engine barriers (preamble and block exit) are elided;
    the runtime's own entry/exit rings already synchronize the engines,
    and the kernel's two streams are fully ordered by data semaphores.
  - No semaphore accumulates across executions: the input DMA's semaphore
    is consumed and then cleared by the DVE, the compute semaphore is
    cleared by SP the moment its wait passes, and the output DMA carries
    no semaphore update at all (the runtime tracks DMA completion on its
    own, and its teardown outlasts the transfer by several microseconds).
  - DMA queue declarations are trimmed 49 -> 3 so queue setup is short.
"""

import numpy as np

import concourse.bass as bass
import concourse.mybir as mybir
from concourse.bass_utils import run_bass_kernel_spmd
from concourse.tile import TileContext

B, C, H, W = 64, 3, 512, 512
PATCH = 8  # top-left patch is 8x8
NUM_CLASSES = 10
N_CORES = 8
PER_CORE = B // N_CORES  # 8 rows per core
D = C * PATCH * PATCH  # 192 reduced elements per row
SCALE = D / 10.0  # t = sum/SCALE; thresholds pre-multiplied by SCALE

_NC = None
LAST_RESULTS = None  # BassKernelResults of the most recent run (for test harness)


def _const_matrix() -> np.ndarray:
    """(PER_CORE, 4*NUM_CLASSES) f32: [LO1 | LO2 | HI1 | HI2] per class, in
    raw-sum units. Column j of the LO half pairs with column j of the HI
    half: out interval j = ind(LO_j <= sum < HI_j) * 10.
    """
    BIG = 1e30  # sentinel: comparison always false
    lo1 = np.array([-1.0] + [float(c) for c in range(1, NUM_CLASSES)])
    hi1 = np.array([float(c + 1) for c in range(NUM_CLASSES)])
    lo2 = np.array([BIG] + [float(c - 11) for c in range(1, NUM_CLASSES)])
    hi2 = np.array([BIG] + [float(c - 10) for c in range(1, NUM_CLASSES)])
    row = np.concatenate([lo1, lo2, hi1, hi2])
    row = np.where(np.abs(row) < 100.0, row * SCALE, row)
    return np.tile(row.astype(np.float32), (PER_CORE, 1))


def _build_nc() -> bass.Bass:
    # Raw Bass (no Tile): explicit semaphores, at most one sem wait per
    # instruction (CoreV2/V3 codegen rejects instructions that accumulate
    # several waits, which Tile's kernel-tail drain does for this shape of
    # kernel).
    #
    # Single input tensor per core: [x patch (192) | const matrix (40)] so
    # there is exactly one input DMA; the reduce takes the one cross-engine
    # wait and the remaining DVE ops rely on sem-guarded program order.
    orig_memset = bass.BassGpSimd.memset
    orig_aeb = bass.Bass.all_engine_barrier
    bass.BassGpSimd.memset = lambda self, *a, **k: None
    bass.Bass.all_engine_barrier = lambda self, sem_only=False: None
    try:
        nc = bass.Bass(name="loc_cls")

        f32 = mybir.dt.float32
        W4 = 4 * NUM_CLASSES
        xp = nc.dram_tensor("xp", (PER_CORE, D + W4), f32, kind="ExternalInput")
        out = nc.dram_tensor(
            "out", (PER_CORE, NUM_CLASSES), f32, kind="ExternalOutput"
        )
        NC = NUM_CLASSES

        with (
            nc.sbuf_tensor([PER_CORE, D + W4], f32) as xt,
            nc.sbuf_tensor([PER_CORE, 1], f32) as s,
            nc.sbuf_tensor([PER_CORE, W4], f32) as S,
            nc.sbuf_tensor([PER_CORE, 2 * NC], f32) as a,
            nc.sbuf_tensor([PER_CORE, NC], f32) as o,
            nc.semaphore() as in_sem,
            nc.semaphore() as vsem,
            nc.semaphore() as out_sem,
            nc.Block() as block,
        ):

            @block.sync
            def _(sync):
                sync.dma_start(out=xt[:], in_=xp[:]).then_inc(in_sem, 16)
                sync.wait_ge(vsem, 4)
                # vsem's four increments have all landed; zero it before the
                # output DMA so no semaphore carries state into the next
                # execution of this NEFF.
                sync.sem_clear(vsem)
                # Nothing waits on the output DMA's completion semaphore
                # (codegen requires one): the runtime tracks pending DMAs
                # itself, and its teardown outlasts the 320-byte transfer by
                # several microseconds.
                sync.dma_start(out=out[:], in_=o[:]).then_inc(out_sem, 16)

            @block.vector
            def _(vector):
                # The DVE is deeply pipelined: a dependent instruction issued
                # back-to-back reads stale data (CoreSim race detector
                # confirms). Every RAW edge below is guarded by a sem
                # inc/wait pair.
                vector.wait_ge(in_sem, 16)
                vector.reduce_sum(
                    out=s[:], in_=xt[:, 0:D], axis=mybir.AxisListType.X
                ).then_inc(vsem, 1)
                vector.wait_ge(vsem, 1)
                # S = (cst <= sum) * 10  -- one fused compare+scale op
                vector.tensor_scalar(
                    out=S[:],
                    in0=xt[:, D : D + W4],
                    scalar1=s[:],
                    scalar2=10.0,
                    op0=mybir.AluOpType.is_le,
                    op1=mybir.AluOpType.mult,
                ).then_inc(vsem, 1)
                vector.wait_ge(vsem, 2)
                # a = 10*(ind(sum >= LO) - ind(sum >= HI)): interval one-hots
                vector.tensor_tensor(
                    out=a[:], in0=S[:, 0 : 2 * NC], in1=S[:, 2 * NC : 4 * NC],
                    op=mybir.AluOpType.subtract,
                ).then_inc(vsem, 1)
                vector.wait_ge(vsem, 3)
                # o = positive-branch + negative-branch interval indicators
                vector.tensor_tensor(
                    out=o[:], in0=a[:, 0:NC], in1=a[:, NC : 2 * NC],
                    op=mybir.AluOpType.add,
                ).then_inc(vsem, 1)
                # in_sem (value 16, consumed by the wait above) returns to 0
                # here, after the last read of xt.
                vector.sem_clear(in_sem)
    finally:
        bass.BassGpSimd.memset = orig_memset
        bass.Bass.all_engine_barrier = orig_aeb

    # PE / Activation only carry dead preamble register-moves; drop them so
    # the compiled NEFF gives those engines nothing to do.
    drop = {mybir.EngineType.PE, mybir.EngineType.Activation}
    for func in nc.m.functions:
        for bb in func.blocks:
            bb.instructions = [i for i in bb.instructions if i.engine not in drop]

    # Declared DMA queues drive the runtime's per-execution queue setup.
    # Default is 3 declarations x 16 queues = ~49 physical queues; this
    # kernel issues exactly two DMAs, both from SP. Keep Pool's SWDGE queue
    # (its engine preamble configures it) and 2 SP HWDGE queues.
    for q in nc.m.queues:
        if q.name == "qPoolDynamic":
            q.num_queues = 1
        elif q.name == "qSPDynamicHW":
            q.num_queues = 2
    nc.m.queues = [q for q in nc.m.queues if q.name != "qActDynamicHW"]

    return nc


def _get_nc() -> bass.Bass:
    global _NC
    if _NC is None:
        _NC = _build_nc()
    return _NC


def kernel(x: np.ndarray) -> np.ndarray:
    global LAST_RESULTS
    x = np.asarray(x)
    assert x.shape == (B, C, H, W), x.shape
    # Host-side sharding: slice out the only live bytes and split by batch.
    patch = x[:, :, :PATCH, :PATCH].astype(np.float32, copy=False).reshape(B, D)
    cst = _const_matrix()
    merged = np.concatenate([patch, np.tile(cst, (N_CORES, 1))], axis=1)
    in_maps = [
        {"xp": np.ascontiguousarray(merged[i * PER_CORE : (i + 1) * PER_CORE])}
        for i in range(N_CORES)
    ]
    res = run_bass_kernel_spmd(_get_nc(), in_maps, core_ids=list(range(N_CORES)))
    LAST_RESULTS = res
    return np.concatenate(
        [res.results[i]["out"] for i in range(N_CORES)], axis=0
    ).astype(np.float32, copy=False)


# revision 14
# speedup vs baseline: 1.1721x; 1.1721x over previous
"""Trainium2 Bass kernel for nn_LocationDependentClassifier.

Reference computation (for full input x of shape (64, 3, 512, 512) f32):
    top_left = x[:, :, :8, :8].mean(axis=(1, 2, 3))          # (64,)
    pred     = mod(trunc(top_left * 10), 10)                 # int in [0, 10)
    logits   = 10 * one_hot(pred, 10)                        # (64, 10) f32

Only the 8x8 top-left patch of each channel is live: 64*3*8*8 floats (48 KiB)
out of 201 MB. Sharding strategy (pure data parallelism per the hint): the
batch dim is split across the 8 cores, and each core is handed exactly the
bytes it needs -- its 8 images' top-left patches, flattened to (8, 192).

On-device per core (all fp32, all on the DVE; 4-op dependency chain):
    s = reduce_sum(patch_row_b)                              # (8, 1)
    S = (CONST <= s) * 10                                    # (8, 40)
    a = S[:, 0:20] - S[:, 20:40]                             # (8, 20)
    o = a[:, 0:10] + a[:, 10:20]                             # (8, 10)

CONST columns are [LO(20) | HI(20)], LO = [lo1 | lo2], HI = [hi1 | hi2], so
a[:, j] = 10 * ind(lo_j <= s < hi_j) and o sums the positive/negative trunc
branches. Class c fires iff t in [c, c+1) (positive branch; c=0 widens to
[-1, 1)) or t in [c-11, c-10) (negative branch, c >= 1), t = sum * 10/192.
Thresholds are pre-multiplied by 192/10 so the comparison runs on the raw
sum. Every intermediate is an exact small integer in fp32; the only
inexactness is the sum itself (boundary margin ~5 orders above fp32 noise).

The kernel is latency-bound: the runtime's fixed per-execution wrapper
(engine wake + clock-sync rings before the program, a full semaphore sweep
after it) dominates, so the controllable cost is the span from the first
program instruction to the last engine's retirement. To keep that span
minimal:
  - Bass's const-AP memset preamble is skipped (nothing reads it).
  - Both bass all-engine barriers (preamble and block exit) are elided;
    the runtime's own entry/exit rings already synchronize the engines,
    and the kernel's two streams are fully ordered by data semaphores.
  - No semaphore accumulates across executions: the input DMA's semaphore
    is consumed and then cleared by the DVE, the compute semaphore is
    cleared by SP the moment its wait passes, and the output DMA carries
    no semaphore update at all (the runtime tracks DMA completion on its
    own, and its teardown outlasts the transfer by several microseconds).
  - DMA queue declarations are trimmed 49 -> 3 so queue setup is short.
"""

import numpy as np

import concourse.bass as bass
import concourse.mybir as mybir
from concourse.bass_utils import run_bass_kernel_spmd
from concourse.tile import TileContext

B, C, H, W = 64, 3, 512, 512
PATCH = 8  # top-left patch is 8x8
NUM_CLASSES = 10
N_CORES = 8
PER_CORE = B // N_CORES  # 8 rows per core
D = C * PATCH * PATCH  # 192 reduced elements per row
SCALE = D / 10.0  # t = sum/SCALE; thresholds pre-multiplied by SCALE

_NC = None
LAST_RESULTS = None  # BassKernelResults of the most recent run (for test harness)


def _const_matrix() -> np.ndarray:
    """(PER_CORE, 4*NUM_CLASSES) f32: [LO1 | LO2 | HI1 | HI2] per class, in
    raw-sum units. Column j of the LO half pairs with column j of the HI
    half: out interval j = ind(LO_j <= sum < HI_j) * 10.
    """
    BIG = 1e30  # sentinel: comparison always false
    lo1 = np.array([-1.0] + [float(c) for c in range(1, NUM_CLASSES)])
    hi1 = np.array([float(c + 1) for c in range(NUM_CLASSES)])
    lo2 = np.array([BIG] + [float(c - 11) for c in range(1, NUM_CLASSES)])
    hi2 = np.array([BIG] + [float(c - 10) for c in range(1, NUM_CLASSES)])
    row = np.concatenate([lo1, lo2, hi1, hi2])
    row = np.where(np.abs(row) < 100.0, row * SCALE, row)
    return np.tile(row.astype(np.float32), (PER_CORE, 1))


def _build_nc() -> bass.Bass:
    # Raw Bass (no Tile): explicit semaphores, at most one sem wait per
    # instruction (CoreV2/V3 codegen rejects instructions that accumulate
    # several waits, which Tile's kernel-tail drain does for this shape of
    # kernel).
    #
    # Single input tensor per core: [x patch (192) | const matrix (40)] so
    # there is exactly one input DMA; the reduce takes the one cross-engine
    # wait and the remaining DVE ops rely on sem-guarded program order.
    orig_memset = bass.BassGpSimd.memset
    orig_aeb = bass.Bass.all_engine_barrier
    bass.BassGpSimd.memset = lambda self, *a, **k: None
    bass.Bass.all_engine_barrier = lambda self, sem_only=False: None
    try:
        nc = bass.Bass(name="loc_cls")

        f32 = mybir.dt.float32
        W4 = 4 * NUM_CLASSES
        xp = nc.dram_tensor("xp", (PER_CORE, D + W4), f32, kind="ExternalInput")
        out = nc.dram_tensor(
            "out", (PER_CORE, NUM_CLASSES), f32, kind="ExternalOutput"
        )
        NC = NUM_CLASSES

        with (
            nc.sbuf_tensor([PER_CORE, D + W4], f32) as xt,
            nc.sbuf_tensor([PER_CORE, 1], f32) as s,
            nc.sbuf_tensor([PER_CORE, W4], f32) as S,
            nc.sbuf_tensor([PER_CORE, 2 * NC], f32) as a,
            nc.sbuf_tensor([PER_CORE, NC], f32) as o,
            nc.semaphore() as in_sem,
            nc.semaphore() as vsem,
            nc.semaphore() as out_sem,
            nc.Block() as block,
        ):

            @block.sync
            def _(sync):
                sync.dma_start(out=xt[:], in_=xp[:]).then_inc(in_sem, 16)
                sync.wait_ge(vsem, 4)
                # Nothing waits on the output DMA's completion semaphore
                # (codegen requires one): the runtime tracks pending DMAs
                # itself, and its teardown outlasts the 320-byte transfer by
                # several microseconds.
                sync.dma_start(out=out[:], in_=o[:]).then_inc(out_sem, 16)
                # Both data semaphores are quiescent now (in_sem at 16 since
                # the input landed, vsem's four increments consumed by the
                # wait above); zero them in one range op so no semaphore
                # carries state into the next execution of this NEFF.
                assert vsem.num == in_sem.num + 1
                sync.sem_clear(range(in_sem.num, vsem.num + 1))

            @block.vector
            def _(vector):
                # The DVE is deeply pipelined: a dependent instruction issued
                # back-to-back reads stale data (CoreSim race detector
                # confirms). Every RAW edge below is guarded by a sem
                # inc/wait pair.
                vector.wait_ge(in_sem, 16)
                vector.reduce_sum(
                    out=s[:], in_=xt[:, 0:D], axis=mybir.AxisListType.X
                ).then_inc(vsem, 1)
                vector.wait_ge(vsem, 1)
                # S = (cst <= sum) * 10  -- one fused compare+scale op
                vector.tensor_scalar(
                    out=S[:],
                    in0=xt[:, D : D + W4],
                    scalar1=s[:],
                    scalar2=10.0,
                    op0=mybir.AluOpType.is_le,
                    op1=mybir.AluOpType.mult,
                ).then_inc(vsem, 1)
                vector.wait_ge(vsem, 2)
                # a = 10*(ind(sum >= LO) - ind(sum >= HI)): interval one-hots
                vector.tensor_tensor(
                    out=a[:], in0=S[:, 0 : 2 * NC], in1=S[:, 2 * NC : 4 * NC],
                    op=mybir.AluOpType.subtract,
                ).then_inc(vsem, 1)
                vector.wait_ge(vsem, 3)
                # o = positive-branch + negative-branch interval indicators
                vector.tensor_tensor(
                    out=o[:], in0=a[:, 0:NC], in1=a[:, NC : 2 * NC],
                    op=mybir.AluOpType.add,
                ).then_inc(vsem, 1)
    finally:
        bass.BassGpSimd.memset = orig_memset
        bass.Bass.all_engine_barrier = orig_aeb

    # PE / Activation only carry dead preamble register-moves; drop them so
    # the compiled NEFF gives those engines nothing to do. The SP/DVE/Pool
    # preamble register-moves (R8=0, R10..R13=-1 defaults) are dead for this
    # kernel too -- nothing reads those registers -- and SP's five sit
    # directly in front of the input DMA on the critical path.
    drop = {mybir.EngineType.PE, mybir.EngineType.Activation}
    for func in nc.m.functions:
        for bb in func.blocks:
            bb.instructions = [
                i
                for i in bb.instructions
                if i.engine not in drop and not isinstance(i, mybir.InstRegisterMove)
            ]

    # Declared DMA queues drive the runtime's per-execution queue setup.
    # Default is 3 declarations x 16 queues = ~49 physical queues; this
    # kernel issues exactly two DMAs, both from SP. Keep Pool's SWDGE queue
    # (its engine preamble configures it) and 2 SP HWDGE queues.
    for q in nc.m.queues:
        if q.name == "qPoolDynamic":
            q.num_queues = 1
        elif q.name == "qSPDynamicHW":
            q.num_queues = 2
    nc.m.queues = [q for q in nc.m.queues if q.name != "qActDynamicHW"]

    return nc


def _get_nc() -> bass.Bass:
    global _NC
    if _NC is None:
        _NC = _build_nc()
    return _NC


def kernel(x: np.ndarray) -> np.ndarray:
    global LAST_RESULTS
    x = np.asarray(x)
    assert x.shape == (B, C, H, W), x.shape
    # Host-side sharding: slice out the only live bytes and split by batch.
    patch = x[:, :, :PATCH, :PATCH].astype(np.float32, copy=False).reshape(B, D)
    cst = _const_matrix()
    merged = np.concatenate([patch, np.tile(cst, (N_CORES, 1))], axis=1)
    in_maps = [
        {"xp": np.ascontiguousarray(merged[i * PER_CORE : (i + 1) * PER_CORE])}
        for i in range(N_CORES)
    ]
    res = run_bass_kernel_spmd(_get_nc(), in_maps, core_ids=list(range(N_CORES)))
    LAST_RESULTS = res
    return np.concatenate(
        [res.results[i]["out"] for i in range(N_CORES)], axis=0
    ).astype(np.float32, copy=False)
